# revision 1
# baseline (speedup 1.0000x reference)
"""ChunkGNNEncoder Trainium kernel v2: host prep + Bass/Tile kernel builder.

Math (per GCN layer, PyG GCNConv):
  h = x @ W              (dense, per-core nodes)
  g = dinv * h           (dinv = deg^-1/2, deg = in-degree incl self-loop)
  agg[t] = sum_{edges (s,t)} g[s]  +  g[t] (self-loop)  +  deg^1/2 * b
  h' = relu(dinv[t] * agg[t])
Then global mean pool per graph, final linear.

v2 design (vs v1):
  - real edges only in the gather/scatter chunks; the self-loop and bias
    terms are added locally into an SBUF fp32 accumulator
  - aggregation accumulates in SBUF (acc[128, NW, 256] f32), one PSUM tile
    per (seg, window) group, DVE-add into acc: segments process
    independently as their AllGather lands (no full-table barrier)
  - 2 source tables (halves; window 0-25 -> half 0), AllGather per half
  - dma_gather with prepare_only=True + trigger_dma so the Pool engine
    never blocks on DMA drain; gb ring gives ~6 batches in flight
  - scales/copies/relu on the Scalar (ACT) engine, st builds and acc adds
    on DVE with a one-batch issue skew so the PE never starves
"""

import numpy as np
import ml_dtypes
from dataclasses import dataclass, field

import concourse.bass as bass
import concourse.bacc as bacc
import concourse.mybir as mybir
import concourse.tile as tile


@dataclass
class Cfg:
    n_nodes: int = 50000
    n_edges: int = 300000
    n_graphs: int = 64
    in_dim: int = 768
    hid: int = 256
    out_dim: int = 128
    n_cores: int = 8
    nw: int = 52          # windows per core (128 nodes each)
    nseg: int = 2         # source table halves; nw % nseg == 0
    rw: int = 4           # windows per aggregation range
    gmax: int = 8         # max chunks per dma_gather call
    use_prep: bool = False  # prepare_only + trigger_dma (broken: tile waits
                            # on DMASW lane sems the prep's sem= never bumps)

    @property
    def p_local(self):
        return self.nw * 128

    @property
    def wseg(self):
        return self.nw // self.nseg

    @property
    def segrows(self):
        return self.wseg * 128

    @property
    def gpc(self):
        return self.n_graphs // self.n_cores

    @property
    def kin(self):
        return self.in_dim // 128

    @property
    def khid(self):
        return self.hid // 128

    @property
    def ranges(self):
        r = []
        w = 0
        while w < self.nw:
            r.append((w, min(w + self.rw, self.nw)))
            w += self.rw
        return r


@dataclass
class Meta:
    C: np.ndarray = None          # [nseg, nw] chunks per (seg, window)
    batch_base: dict = field(default_factory=dict)  # (q, w0) -> (base, nchk)
    tot_chunks: int = 0
    cbmax: int = 0


def host_prep(x, edge_index, batch, W1, b1, W2, b2, Wp, bp, cfg: Cfg):
    N, E, G = cfg.n_nodes, cfg.n_edges, cfg.n_graphs
    NC, NW, NSEG = cfg.n_cores, cfg.nw, cfg.nseg
    src = np.asarray(edge_index[0], dtype=np.int64)
    tgt = np.asarray(edge_index[1], dtype=np.int64)
    batch = np.asarray(batch, dtype=np.int64)

    deg = np.bincount(tgt, minlength=N).astype(np.float64) + 1.0
    dinv = (1.0 / np.sqrt(deg)).astype(np.float32)
    dinvinv = np.sqrt(deg).astype(np.float32)

    node_core = batch // cfg.gpc
    core_nodes = [np.nonzero(node_core == c)[0] for c in range(NC)]
    for c in range(NC):
        assert len(core_nodes[c]) <= cfg.p_local

    indeg = np.bincount(tgt, minlength=N) + 1

    # window bin-packing per core: balance in-degree, <=128 nodes/window
    local_row = np.full(N, -1, np.int64)
    node_window = np.full(N, -1, np.int64)   # window only; slots assigned later
    for c in range(NC):
        nodes = core_nodes[c]
        order = np.argsort(-indeg[nodes], kind="stable")
        wload = np.zeros(NW, np.int64)
        wcount = np.zeros(NW, np.int64)
        win_of = np.empty(len(nodes), np.int64)
        for i in order:
            open_w = np.nonzero(wcount < 128)[0]
            w = open_w[np.argmin(wload[open_w])]
            win_of[i] = w
            wcount[w] += 1
            wload[w] += indeg[nodes[i]]
        node_window[nodes] = win_of

    # --- per-source-half in-degree refinement: source half = source's
    # window-half on its home core (w < wseg -> half 0). Swapping two nodes
    # between windows of the SAME half on the same core moves only their
    # in-edge counts (target side); source-half memberships are unchanged,
    # so the refinement has purely local effect. Goal: per (core, half q,
    # window w) in-edge count <= 3*128 so every chunk group bakes C=3.
    wseg_w = NW // NSEG
    src_half = node_window[src] // wseg_w            # per-edge source half
    # d[q] per target node
    dq = np.zeros((NSEG, N), np.int64)
    for q in range(NSEG):
        np.add.at(dq[q], tgt[src_half == q], 1)
    cap = 3 * 128
    rng = np.random.default_rng(0)
    for c in range(NC):
        nodes = core_nodes[c]
        for hw in range(NSEG):                       # window-half being packed
            wlo, whi = hw * wseg_w, (hw + 1) * wseg_w
            sel = nodes[(node_window[nodes] >= wlo) & (node_window[nodes] < whi)]
            win = node_window[sel] - wlo             # [n] in 0..wseg_w
            d = dq[:, sel]                           # [NSEG, n]
            # loads[q, w]
            loads = np.zeros((NSEG, wseg_w), np.int64)
            for q in range(NSEG):
                np.add.at(loads[q], win, d[q])
            for _ in range(4000):
                worst = np.unravel_index(np.argmax(loads), loads.shape)
                q0, w0 = int(worst[0]), int(worst[1])
                if loads[q0, w0] <= cap - 4:
                    break
                # move a q0-heavy node from w0 to the lightest window,
                # swapping with a q0-light node there (keeps counts at 128)
                w1 = int(np.argmin(loads[q0]))
                if w1 == w0:
                    break
                cand0 = np.nonzero(win == w0)[0]
                cand1 = np.nonzero(win == w1)[0]
                if len(cand0) == 0 or len(cand1) == 0:
                    break
                bias0 = d[q0, cand0] - d[1 - q0, cand0]
                bias1 = d[q0, cand1] - d[1 - q0, cand1]
                i0 = cand0[np.argmax(bias0)]
                i1 = cand1[np.argmin(bias1)]
                gain = d[q0, i0] - d[q0, i1]
                if gain <= 0:
                    break
                win[i0], win[i1] = w1, w0
                for q in range(NSEG):
                    loads[q, w0] += d[q, i1] - d[q, i0]
                    loads[q, w1] += d[q, i0] - d[q, i1]
            node_window[sel] = win + wlo

    # assign slots within windows
    for c in range(NC):
        nodes = core_nodes[c]
        wcount = np.zeros(NW, np.int64)
        lr = np.empty(len(nodes), np.int64)
        wins = node_window[nodes]
        for i in range(len(nodes)):
            w = wins[i]
            lr[i] = w * 128 + wcount[w]
            wcount[w] += 1
        local_row[nodes] = lr

    # --- real edges only, bucketed by (target core, source half, window)
    e_core = node_core[tgt]
    e_lrow_t = local_row[tgt]
    e_w = e_lrow_t // 128
    e_tshift = e_lrow_t % 128
    s_core = node_core[src]
    s_lrow = local_row[src]
    e_seg = s_lrow // cfg.segrows          # source window-half
    e_idx16 = s_core * cfg.segrows + (s_lrow - e_seg * cfg.segrows)
    assert e_idx16.max() < 32768

    cnt = np.zeros((NC, NSEG, NW), np.int64)
    np.add.at(cnt, (e_core, e_seg, e_w), 1)
    C = np.maximum(0, -(-cnt.max(axis=0) // 128))     # [NSEG, NW]

    meta = Meta()
    meta.C = C

    # chunk order: seg-major -> range -> window -> chunks
    chunk_order = []
    batch_base = {}
    for q in range(NSEG):
        for (w0, w1) in cfg.ranges:
            base = len(chunk_order)
            for w in range(w0, w1):
                for _ in range(C[q, w]):
                    chunk_order.append((q, w))
            batch_base[(q, w0)] = (base, len(chunk_order) - base)
    meta.batch_base = batch_base
    meta.tot_chunks = len(chunk_order)
    meta.cbmax = max((n for (_, n) in batch_base.values()), default=1)
    TOTC = meta.tot_chunks
    TOTS = TOTC * 128

    slot_base = {}
    pos = 0
    for q in range(NSEG):
        for (w0, w1) in cfg.ranges:
            for w in range(w0, w1):
                slot_base[(q, w)] = pos
                pos += C[q, w] * 128
    assert pos == TOTS

    dt_bf16 = ml_dtypes.bfloat16
    in_maps = []
    W1b = np.asarray(W1, np.float32).astype(dt_bf16)
    W2b = np.asarray(W2, np.float32).astype(dt_bf16)
    Wpb = np.asarray(Wp, np.float32).astype(dt_bf16)
    b1f = np.asarray(b1, np.float32)
    b2f = np.asarray(b2, np.float32)
    bpf = np.asarray(bp, np.float32).reshape(1, -1)
    b1_bcast = np.tile(b1f[None, :], (128, 1))        # [128, HID]
    b2_bcast = np.tile(b2f[None, :], (128, 1))
    x = np.asarray(x, np.float32)

    iota_rep = np.tile(np.arange(128, dtype=np.int8)[None, :],
                       (128, max(meta.cbmax, 1)))

    for c in range(NC):
        mask = e_core == c
        cs, cw, ct, cq = (e_seg[mask], e_w[mask], e_tshift[mask],
                          e_idx16[mask])
        idx_flat = np.zeros(TOTS, np.int64)
        tsh_flat = np.full(TOTS, -1, np.int64)
        key = cs * NW + cw
        order = np.argsort(key, kind="stable")
        ks, kt, kq2, kw = cq[order], ct[order], cs[order], cw[order]
        uniq, starts = np.unique(kq2 * NW + kw, return_index=True)
        starts = list(starts) + [len(ks)]
        for u, s0, s1 in zip(uniq, starts[:-1], starts[1:]):
            q, w = int(u) // NW, int(u) % NW
            n = s1 - s0
            b = slot_base[(q, w)]
            assert n <= C[q, w] * 128
            so = np.argsort(ks[s0:s1], kind="stable")
            idx_flat[b:b + n] = ks[s0:s1][so]
            tsh_flat[b:b + n] = kt[s0:s1][so]

        idxp = idx_flat.reshape(-1, 16).T.astype(np.int16)
        idxp = np.tile(idxp, (8, 1))                  # [128, TOTS/16]
        tshp = tsh_flat.reshape(TOTC, 128).T.astype(np.int8)

        nodes = core_nodes[c]
        xT = np.zeros((cfg.in_dim, cfg.p_local), np.float32)
        xT[:, local_row[nodes]] = x[nodes].T
        xTb = xT.astype(dt_bf16)

        dinv_l = np.zeros(cfg.p_local, np.float32)
        dinv_l[local_row[nodes]] = dinv[nodes]
        dinvinv_l = np.zeros(cfg.p_local, np.float32)
        dinvinv_l[local_row[nodes]] = dinvinv[nodes]
        dinv_cols = dinv_l.reshape(NW, 128).T.copy()      # [128, NW]
        dinvinv_cols = dinvinv_l.reshape(NW, 128).T.copy()

        spool = np.zeros((cfg.p_local, cfg.gpc), np.float32)
        gl = batch[nodes] - c * cfg.gpc
        spool[local_row[nodes], gl] = 1.0
        spool = (spool.reshape(NW, 128, cfg.gpc).transpose(1, 0, 2)
                 .reshape(128, NW * cfg.gpc)).astype(dt_bf16)
        cnt_g = np.bincount(gl, minlength=cfg.gpc).astype(np.float32)
        cntinv = (1.0 / np.maximum(cnt_g, 1.0)).astype(np.float32)
        cntinv_rep = np.tile(cntinv[None, :], (128, cfg.khid))

        ident = np.eye(128, dtype=dt_bf16)

        in_maps.append(dict(
            xT=xTb, W1=W1b, W2=W2b, Wp=Wpb,
            b1b=b1_bcast, b2b=b2_bcast,
            bp8=np.tile(bpf, (cfg.gpc, 1)).astype(np.float32),
            dinv_cols=dinv_cols, dinvinv_cols=dinvinv_cols,
            idx=idxp, tsh=tshp, iota=iota_rep, spool=spool,
            cntinv=cntinv_rep, ident=ident,
        ))
    return in_maps, meta


def build_kernel(cfg: Cfg, meta: Meta, debug=False):
    NC, NW, NSEG = cfg.n_cores, cfg.nw, cfg.nseg
    HID, OUT, GPC = cfg.hid, cfg.out_dim, cfg.gpc
    KIN, KHID = cfg.kin, cfg.khid
    C = meta.C
    TOTC = meta.tot_chunks
    TOTS = TOTC * 128
    CBMAX = meta.cbmax
    bf16, f32 = mybir.dt.bfloat16, mybir.dt.float32
    tabrows = NC * cfg.segrows
    WSEG = cfg.wseg
    Relu = mybir.ActivationFunctionType.Relu

    nc = bacc.Bacc(None, target_bir_lowering=False, debug=debug,
                   num_devices=NC if NC > 1 else None,
                   num_swdge_queues=4)

    dram_in = lambda n, s, d: nc.dram_tensor(n, s, d, kind="ExternalInput")
    xT_d = dram_in("xT", [cfg.in_dim, cfg.p_local], bf16)
    W1_d = dram_in("W1", [cfg.in_dim, HID], bf16)
    W2_d = dram_in("W2", [HID, HID], bf16)
    Wp_d = dram_in("Wp", [HID, OUT], bf16)
    b1b_d = dram_in("b1b", [128, HID], f32)
    b2b_d = dram_in("b2b", [128, HID], f32)
    bp8_d = dram_in("bp8", [GPC, OUT], f32)
    dinv_d = dram_in("dinv_cols", [128, NW], f32)
    dinvinv_d = dram_in("dinvinv_cols", [128, NW], f32)
    idx_d = dram_in("idx", [128, TOTS // 16], mybir.dt.int16)
    tsh_d = dram_in("tsh", [128, TOTC], mybir.dt.int8)
    iota_d = dram_in("iota", [128, 128 * CBMAX], mybir.dt.int8)
    spool_d = dram_in("spool", [128, NW * GPC], bf16)
    cntinv_d = dram_in("cntinv", [128, KHID * GPC], f32)
    ident_d = dram_in("ident", [128, 128], bf16)
    out_d = nc.dram_tensor("out", [GPC, OUT], f32, kind="ExternalOutput")

    dma_sems = [nc.alloc_semaphore(f"swdge{i}") for i in range(4)]
    sem_i = [0]

    with tile.TileContext(nc) as tc:
        with (
            tc.tile_pool(name="const", bufs=1) as cpool,
            tc.tile_pool(name="xw", bufs=3) as xwpool,
            tc.tile_pool(name="gbuf", bufs=8) as gpool,
            tc.tile_pool(name="stb", bufs=4) as stpool,
            tc.tile_pool(name="flush", bufs=4) as fpool,
            tc.tile_pool(name="psagg", bufs=5, space="PSUM") as psagg,
            tc.tile_pool(name="psx", bufs=2, space="PSUM") as psx,
            tc.tile_pool(name="pspool", bufs=1, space="PSUM") as pspool,
            tc.tile_pool(name="dram", bufs=1, space="DRAM") as dram,
        ):
            # ---- constants
            W1_t = cpool.tile([128, KIN, HID], bf16)
            nc.sync.dma_start(W1_t[:], W1_d[:].rearrange("(k p) n -> p k n", p=128))
            W2_t = cpool.tile([128, KHID, HID], bf16)
            nc.sync.dma_start(W2_t[:], W2_d[:].rearrange("(k p) n -> p k n", p=128))
            Wp_t = cpool.tile([128, KHID, OUT], bf16)
            nc.sync.dma_start(Wp_t[:], Wp_d[:].rearrange("(k p) n -> p k n", p=128))
            b1b_t = cpool.tile([128, HID], f32)
            nc.sync.dma_start(b1b_t[:], b1b_d[:])
            b2b_t = cpool.tile([128, HID], f32)
            nc.sync.dma_start(b2b_t[:], b2b_d[:])
            bp8_t = cpool.tile([GPC, OUT], f32)
            nc.sync.dma_start(bp8_t[:], bp8_d[:])
            dinv_t = cpool.tile([128, NW], f32)
            nc.sync.dma_start(dinv_t[:], dinv_d[:])
            dinvinv_t = cpool.tile([128, NW], f32)
            nc.sync.dma_start(dinvinv_t[:], dinvinv_d[:])
            idx_t = cpool.tile([128, TOTS // 16], mybir.dt.int16)
            nc.sync.dma_start(idx_t[:], idx_d[:])
            tsh_t = cpool.tile([128, TOTC], mybir.dt.int8)
            nc.sync.dma_start(tsh_t[:], tsh_d[:])
            iota_t = cpool.tile([128, 128 * CBMAX], mybir.dt.int8)
            nc.sync.dma_start(iota_t[:], iota_d[:])
            spool_t = cpool.tile([128, NW * GPC], bf16)
            nc.sync.dma_start(spool_t[:], spool_d[:])
            cntinv_t = cpool.tile([128, KHID * GPC], f32)
            nc.sync.dma_start(cntinv_t[:], cntinv_d[:])
            ident_t = cpool.tile([128, 128], bf16)
            nc.sync.dma_start(ident_t[:], ident_d[:])

            # persistent SBUF state
            acc_t = cpool.tile([128, NW, HID], f32)       # 53KB/part
            g_sb = cpool.tile([128, NW, HID], bf16)       # 26.6KB/part

            # ---- AG tables (DRAM)
            ag_in = [[dram.tile([cfg.segrows, HID], bf16, tag=f"agin{l}{q}",
                                name=f"agin{l}{q}")
                      for q in range(NSEG)] for l in range(2)]
            ag_out = [[dram.tile([tabrows, HID], bf16, tag=f"agout{l}{q}",
                                 name=f"agout{l}{q}",
                                 addr_space="Shared" if NC > 1 else "Local")
                       for q in range(NSEG)] for l in range(2)]

            def launch_ag(layer, q, j=None):
                """AllGather half q; j in {0,1} gathers quarter j only
                (strided output rows c*segrows + [j*half..])."""
                if NC == 1:
                    return
                if j is None:
                    ins = ag_in[layer][q][:]
                    outs = ag_out[layer][q][:]
                else:
                    hr = cfg.segrows // 2
                    ins = ag_in[layer][q][j * hr:(j + 1) * hr, :]
                    outs = (ag_out[layer][q][:]
                            .rearrange("(c s) f -> c s f", s=cfg.segrows)
                            [:, j * hr:(j + 1) * hr, :])
                nc.gpsimd.collective_compute(
                    "AllGather", mybir.AluOpType.bypass,
                    replica_groups=[list(range(NC))],
                    ins=[ins.opt()],
                    outs=[outs.opt()],
                )

            def g_write(layer, w):
                q, wr = w // WSEG, w % WSEG
                dst = ag_out[layer][q] if NC == 1 else ag_in[layer][q]
                nc.sync.dma_start(dst[wr * 128:(wr + 1) * 128, :],
                                  g_sb[:, w, :])

            # gather preps for one (q, range) batch
            def issue_batch_gather(layer, q, w0):
                base, nchk = meta.batch_base[(q, w0)]
                if nchk == 0:
                    return None
                gb = gpool.tile([128, CBMAX, HID], bf16, tag="gb")
                ncalls = -(-nchk // cfg.gmax)
                per = -(-nchk // ncalls)
                for g0 in range(0, nchk, per):
                    g1 = min(g0 + per, nchk)
                    if cfg.use_prep:
                        sem = dma_sems[sem_i[0] % len(dma_sems)]
                        sem_i[0] += 1
                        nc.gpsimd.dma_gather(
                            gb[:, g0:g1, :], ag_out[layer][q][:],
                            idx_t[:, (base + g0) * 8:(base + g1) * 8],
                            num_idxs=(g1 - g0) * 128,
                            num_idxs_reg=(g1 - g0) * 128,
                            elem_size=HID,
                            prepare_only=True, sem=sem)
                        nc.gpsimd.trigger_dma(count=None)
                    else:
                        nc.gpsimd.dma_gather(
                            gb[:, g0:g1, :], ag_out[layer][q][:],
                            idx_t[:, (base + g0) * 8:(base + g1) * 8],
                            num_idxs=(g1 - g0) * 128,
                            num_idxs_reg=(g1 - g0) * 128,
                            elem_size=HID,
                            queue_num=sem_i[0] % 4)
                        sem_i[0] += 1
                return gb

            def build_st(q, w0):
                base, nchk = meta.batch_base[(q, w0)]
                if nchk == 0:
                    return None
                st = stpool.tile([128, CBMAX * 128], bf16, tag="st")
                nc.vector.tensor_tensor(
                    out=st[:, :nchk * 128].rearrange("p (c i) -> p c i", i=128),
                    in0=tsh_t[:, base:base + nchk].unsqueeze(2)
                        .broadcast_to([128, nchk, 128]),
                    in1=iota_t[:, :nchk * 128].rearrange("p (c i) -> p c i", i=128),
                    op=mybir.AluOpType.is_equal)
                return st

            def agg_batch_mms(q, w0, w1, gb, st):
                ci = 0
                for w in range(w0, w1):
                    cq = int(C[q, w])
                    if cq == 0:
                        continue
                    ps = psagg.tile([128, HID], f32, tag="psagg")
                    for k in range(cq):
                        nc.tensor.matmul(ps[:], st[:, ci * 128:(ci + 1) * 128],
                                         gb[:, ci, :],
                                         start=(k == 0), stop=(k == cq - 1))
                        ci += 1
                    nc.vector.tensor_tensor(out=acc_t[:, w, :], in0=ps[:],
                                            in1=acc_t[:, w, :],
                                            op=mybir.AluOpType.add)

            # =================== L1 dense + acc init =====================
            for w in range(NW):
                xw = xwpool.tile([128, KIN, 128], bf16, tag="xw")
                nc.sync.dma_start(
                    xw[:],
                    xT_d[:, w * 128:(w + 1) * 128]
                    .rearrange("(k p) n -> p k n", p=128))
                psd = psx.tile([128, HID], f32, tag="psx")
                for k in range(KIN):
                    nc.tensor.matmul(psd[:], xw[:, k, :], W1_t[:, k, :],
                                     start=(k == 0), stop=(k == KIN - 1))
                nc.scalar.mul(g_sb[:, w, :], psd[:], dinv_t[:, w:w + 1])
                g_write(0, w)
                # acc init: deg^1/2 * b + self-loop g
                nc.scalar.mul(acc_t[:, w, :], b1b_t[:], dinvinv_t[:, w:w + 1])
                nc.vector.tensor_tensor(out=acc_t[:, w, :], in0=acc_t[:, w, :],
                                        in1=g_sb[:, w, :],
                                        op=mybir.AluOpType.add)
                if (w + 1) % WSEG == 0:
                    launch_ag(0, w // WSEG)

            # =================== generic agg phase =======================
            def flush_d(w):
                """L1 flush -> L2 dense -> g2 -> acc2 init for one window."""
                hp = fpool.tile([128, HID], bf16, tag="hflush")
                nc.scalar.activation(hp[:], acc_t[:, w, :], Relu,
                                     scale=dinv_t[:, w:w + 1])
                xt2 = fpool.tile([128, KHID, 128], bf16, tag="xt2")
                for h in range(KHID):
                    pt = psx.tile([128, 128], bf16, tag="psx")
                    nc.tensor.transpose(pt[:], hp[:, h * 128:(h + 1) * 128],
                                        ident_t[:])
                    nc.scalar.copy(xt2[:, h, :], pt[:])
                ps2 = psx.tile([128, HID], f32, tag="psx")
                for k in range(KHID):
                    nc.tensor.matmul(ps2[:], xt2[:, k, :], W2_t[:, k, :],
                                     start=(k == 0), stop=(k == KHID - 1))
                nc.scalar.mul(g_sb[:, w, :], ps2[:], dinv_t[:, w:w + 1])
                g_write(1, w)
                nc.scalar.mul(acc_t[:, w, :], b2b_t[:], dinvinv_t[:, w:w + 1])
                nc.vector.tensor_tensor(out=acc_t[:, w, :], in0=acc_t[:, w, :],
                                        in1=g_sb[:, w, :],
                                        op=mybir.AluOpType.add)
                if (w + 1) % WSEG == 0:
                    launch_ag(1, w // WSEG)

            pooled = pspool.tile([128, KHID * GPC], f32)
            zrow_t = cpool.tile([1, KHID * GPC], bf16)
            nc.vector.memset(zrow_t[:], 0.0)
            # single zeroing init for the whole pooled bank: a later start
            # would wipe the full bank region, clobbering the sibling group
            nc.tensor.matmul(pooled[:], ident_t[0:1, :], zrow_t[:],
                             start=True, stop=False, skip_group_check=True)

            def flush_f(w):
                """L2 flush -> pool matmuls for one window."""
                hp = fpool.tile([128, HID], bf16, tag="hflush")
                nc.scalar.activation(hp[:], acc_t[:, w, :], Relu,
                                     scale=dinv_t[:, w:w + 1])
                for h in range(KHID):
                    nc.tensor.matmul(
                        pooled[:, h * GPC:(h + 1) * GPC],
                        hp[:, h * 128:(h + 1) * 128],
                        spool_t[:, w * GPC:(w + 1) * GPC],
                        start=False,
                        stop=(w == NW - 1 and h == KHID - 1),
                        skip_group_check=True)

            def agg_phase(layer, flush_fn):
                batches = [(q, w0, w1) for q in range(NSEG)
                           for (w0, w1) in cfg.ranges]
                n = len(batches)
                gbs = [None] * n
                sts = [None] * n
                # skew: st + gather for batch i issued before mms of batch i-1
                gbs[0] = issue_batch_gather(layer, batches[0][0], batches[0][1])
                sts[0] = build_st(batches[0][0], batches[0][1])
                for i in range(n):
                    if i + 1 < n:
                        q2, w02, _ = batches[i + 1]
                        gbs[i + 1] = issue_batch_gather(layer, q2, w02)
                        sts[i + 1] = build_st(q2, w02)
                    q, w0, w1 = batches[i]
                    if gbs[i] is not None:
                        agg_batch_mms(q, w0, w1, gbs[i], sts[i])
                    # second half (q == NSEG-1): windows are complete; flush
                    if q == NSEG - 1 and flush_fn is not None:
                        for w in range(w0, w1):
                            flush_fn(w)

            agg_phase(0, flush_d)
            agg_phase(1, flush_f)

            # =================== pooled -> mean -> final =================
            pooledT = fpool.tile([128, KHID * GPC], bf16, tag="pooledT")
            nc.vector.tensor_tensor(out=pooledT[:], in0=pooled[:],
                                    in1=cntinv_t[:], op=mybir.AluOpType.mult)
            ps_out = psx.tile([GPC, OUT], f32, tag="psx")
            for k in range(KHID):
                nc.tensor.matmul(ps_out[:],
                                 pooledT[:, k * GPC:(k + 1) * GPC],
                                 Wp_t[:, k, :],
                                 start=(k == 0), stop=(k == KHID - 1))
            out_sb = fpool.tile([GPC, OUT], f32, tag="outsb")
            nc.vector.tensor_tensor(out=out_sb[:], in0=ps_out[:],
                                    in1=bp8_t[:], op=mybir.AluOpType.add)
            nc.sync.dma_start(out_d[:], out_sb[:])

    nc.compile()
    return nc


def kernel(**inputs) -> "np.ndarray":
    cfg = Cfg()
    in_maps, meta = host_prep(
        inputs["x"], inputs["edge_index"], inputs["batch"],
        inputs["W1"], inputs["b1"], inputs["W2"], inputs["b2"],
        inputs["Wp"], inputs["bp"], cfg)
    nc = build_kernel(cfg, meta, debug=False)
    from concourse.bass_utils import run_bass_kernel_spmd
    res = run_bass_kernel_spmd(nc, in_maps,
                               core_ids=list(range(cfg.n_cores)), trace=False)
    out = np.concatenate([r["out"] for r in res.results], axis=0)
    return np.ascontiguousarray(out.astype(np.float32))

